# revision 35
# baseline (speedup 1.0000x reference)
"""MinGRU Trainium2 kernel.

Problem: x (8, 4096, 1024) fp32; Wz, Wh (1024, 1024); bz, bh (1024,).
    k = x @ Wz.T + bz ; z = sigmoid(k)
    p = x @ Wh.T + bh ; g = where(p >= 0, p + 0.5, sigmoid(p))
    h_t = (1 - z_t) * h_{t-1} + z_t * g_t   (h_0 = 0.5)
The reference computes this recurrence with a log-space parallel scan; here it
is computed directly in linear space (mathematically identical), using the DVE
TensorTensorScanArith instruction along the free axis.

Sharding: data-parallel over batch, one batch element per NeuronCore (8 cores).

Per-core layout: everything lives transposed, H on partitions, S on the free
axis.  The two GEMMs run in fp8 e4m3 with perf_mode=DoubleRow (2 weights per
PE cell, K=256 per matmul -> half the matmul count of full-rate fp32).  Inputs
are quantized host-side with power-of-two scales (x*16, W*1024); the exact
descale 2^-14 is folded into the ScalarE activation `scale` argument.

Work is chunked in 1024-wide sequence units (PSUM tiles span 2 banks) to
amortize the per-instruction overheads (~352 cycles per ACT, ~200 per DVE op)
and halve the semaphore traffic.  Engine assignment (all rates measured on
hardware and fixed per 128-partition column: ACT 1.09, DVE stt 1.11 /
tt 0.67 / scan 2.23 ns/col regardless of dtype):
    ScalarE: sp = sigmoid(pp), rp = relu(pp), a = sigmoid(-kp)  (bias fused)
    DVE:     g = min(sp,.5)+rp ; b_neg = (a-1)*g ; h = scan(a, -b_neg)
GpSimd must stay COMPLETELY idle: any concurrent Pool compute multiplies DVE
op latency ~5x (measured: stt 1.11 -> 6.1 ns/col) and Pool ops themselves run
3-15 ns/col.  Materializing z via a 4th ACT to turn the b-stt into a tt-mult
saves ~7 us of DVE op time but adds ~11 us of cross-engine semaphore stalls —
measured net loss.  The scan is issued one (unit, m) slot late AND ahead of g
in program order: engine queues are strict FIFO, so the op at the DVE head
must always have ready inputs.  Gate tensors and h are uniform bf16 (same DVE
speed as uniform fp32, halves the output DMA; mixed dtypes are ~2x slower).
DVE total (~147 us ops + ~8 us sems) is the critical resource, above the PE
stream (~124 us incl. per-instruction overhead) and ScalarE (~108 us).
Startup: DMA triggers cost ~640 ns each and transfers serialize, so the
unit-0 operands load in strict need-order on the Sync queue (big transfers
triggered from the Scalar queue stall the first ACTs ~7 us); 26 dummy
matmuls + 2 dummy ACTs keep the PE clock at 2.4 GHz (HAM gate) and preload
the activation tables before the first real slot.  Measured end-to-end
rel-err ~1.33e-2 against the fp32 reference, within 2e-2.
"""

import os
import sys

import numpy as np

for _p in ("/opt/trn_rl_repo", "/root/.axon_site/_ro/trn_rl_repo"):
    if os.path.isdir(_p) and _p not in sys.path:
        sys.path.insert(0, _p)

import ml_dtypes  # noqa: E402

import concourse.bass as bass  # noqa: E402
import concourse.mybir as mybir  # noqa: E402
import concourse.tile as tile  # noqa: E402
from concourse import bacc  # noqa: E402
from concourse.bass_utils import run_bass_kernel_spmd  # noqa: E402

F32 = mybir.dt.float32
F32R = mybir.dt.float32r
BF16 = mybir.dt.bfloat16
F8 = mybir.dt.float8e4  # TRN e4m3 (bias 8, max +-240) == ml_dtypes.float8_e4m3
NP_F8 = ml_dtypes.float8_e4m3
NP_BF16 = ml_dtypes.bfloat16
N_CORES = 8
B, S, D, H = 8, 4096, 1024, 1024
NK = D // 128  # 8 k-tiles of 128
NKP = NK // 2  # 4 DoubleRow k-pairs
NM = H // 128

# power-of-two quantization scales; descale folded into the activations
SX = 16.0
SW = 1024.0
DESCALE = 1.0 / (SX * SW)

_cache: dict = {}


def build_nc(seq_len: int = S, n_cores: int = N_CORES):
    """Build and compile the per-core Bass module (SPMD, identical program)."""
    tsp = min(1024, seq_len)  # strip width (2 PSUM banks of fp32 at 1024)
    nst = seq_len // tsp
    nc = bacc.Bacc(
        "TRN2", target_bir_lowering=False, debug=False, num_devices=n_cores
    )

    # x packed host-side as [p, strip, ktile, t] so one DMA fetches a strip
    xp_d = nc.dram_tensor("xp8", [128, nst, NK, tsp], F8, kind="ExternalInput")
    # weights packed as [p, ktile, m] (wz8[p, kt, m] = Wz[m, kt*128+p] * SW),
    # split into m<512 / m>=512 halves so each half is one contiguous DMA and
    # the two halves can load on different queues without same-tile ordering.
    wzlo_d = nc.dram_tensor("wz8lo", [128, NK, H // 2], F8, kind="ExternalInput")
    wzhi_d = nc.dram_tensor("wz8hi", [128, NK, H // 2], F8, kind="ExternalInput")
    whlo_d = nc.dram_tensor("wh8lo", [128, NK, H // 2], F8, kind="ExternalInput")
    whhi_d = nc.dram_tensor("wh8hi", [128, NK, H // 2], F8, kind="ExternalInput")
    bzn_d = nc.dram_tensor("bzn", [H], F32, kind="ExternalInput")
    bh_d = nc.dram_tensor("bh", [H], F32, kind="ExternalInput")
    hT_d = nc.dram_tensor("hT", [H, seq_len], BF16, kind="ExternalOutput")

    AF = mybir.ActivationFunctionType
    OP = mybir.AluOpType
    DR = mybir.MatmulPerfMode.DoubleRow

    with tile.TileContext(nc) as tc:
        with (
            tc.tile_pool(name="singles", bufs=1) as singles,
            tc.tile_pool(name="xs", bufs=3) as xpool,
            tc.tile_pool(name="work", bufs=3) as work,
            tc.tile_pool(name="hbuf", bufs=2) as hpool,
            tc.tile_pool(name="psum", bufs=2, space="PSUM") as psum,
        ):
            # PE warm-up: the HAM clock gate holds the PE at 1.2 GHz until it
            # has been busy ~3.4 us.  The PE sits idle anyway while the first
            # DMAs land, so burn that time on dummy matmuls over a zeroed
            # tile — the first real matmuls then run at 2.4 GHz.
            warm = singles.tile([128, 256], F32, tag="warm")
            nc.gpsimd.memset(warm[:], 0.0)
            # Pull the lazy ACT_TABLE_LOAD (~1.3 us) off the critical path:
            # a 1-column dummy activation right at program start loads the
            # sigmoid/relu tables while the input DMAs are still in flight.
            warm_act = singles.tile([128, 1], F32, tag="warmact")
            nc.scalar.activation(out=warm_act[:], in_=warm[:, :1],
                                 func=AF.Sigmoid)
            nc.scalar.activation(out=warm_act[:], in_=warm[:, :1],
                                 func=AF.Relu)
            # 26 dummy matmuls keep the PE busy until the first real operands
            # land (~12 us in): the HAM clock gate holds the PE at 1.2 GHz
            # until it has been busy ~3.4 us, and it falls back to 1.2 GHz if
            # the PE goes idle again before the first real matmuls.
            wps = psum.tile([128, tsp], F32, tag="kp")
            for i in range(26):
                nc.tensor.matmul(
                    wps[:, :256], lhsT=warm[:, :128].bitcast(F32R),
                    rhs=warm[:].bitcast(F32R),
                    start=(i == 0), stop=(i == 25),
                )
            # Startup loads (see module docstring): strict need-order on the
            # Sync queue; pp is computed before kp (sp/rp gate the first DVE
            # op), so wh-low precedes wz-low.
            xs0 = xpool.tile([128, NK, tsp], F8, tag="xs")
            wz_lo = singles.tile([128, NK, H // 2], F8, tag="wzlo")
            wz_hi = singles.tile([128, NK, H // 2], F8, tag="wzhi")
            wh_lo = singles.tile([128, NK, H // 2], F8, tag="whlo")
            wh_hi = singles.tile([128, NK, H // 2], F8, tag="whhi")
            bh_sb = singles.tile([128, NM], F32, tag="bh")
            bzn_sb = singles.tile([128, NM], F32, tag="bzn")
            # All startup loads ride the Sync queue in exact need-order —
            # transfers serialize on the DMA engine anyway, and any big
            # transfer triggered from the Scalar queue blocks the first ACTs
            # behind it.  Tiny biases first (they gate every ACT), then the
            # unit-0 operands in the order the PE consumes them.
            nc.sync.dma_start(out=bh_sb,
                              in_=bh_d.ap().rearrange("(m p) -> p m", p=128))
            nc.sync.dma_start(out=bzn_sb,
                              in_=bzn_d.ap().rearrange("(m p) -> p m", p=128))
            nc.sync.dma_start(out=xs0[:, 0:2, :], in_=xp_d.ap()[:, 0, 0:2, :])
            nc.sync.dma_start(out=wh_lo, in_=whlo_d.ap())
            nc.sync.dma_start(out=xs0[:, 2:, :], in_=xp_d.ap()[:, 0, 2:, :])
            nc.sync.dma_start(out=wz_lo, in_=wzlo_d.ap())
            hi_loaded = [False]
            # Sequence units: full strips of `tsp`, with the final strip split
            # in half so the end-of-kernel pipeline drain runs on narrower
            # tiles.
            units = [(s, 0, tsp) for s in range(nst - 1)]
            units += [(nst - 1, 0, tsp // 2), (nst - 1, tsp // 2, tsp // 2)]
            h_prev: list = [None] * NM
            pending: list = []

            pair: dict = {}

            def gate_front(m, kp, pp, tw, ts_sl):
                """ScalarE + DVE gate math for one (unit, m) slot.

                DVE is the critical engine (stt 1.11 / tt 0.67 / scan 2.23
                ns/col are fixed per-column rates; GpSimd is unusable — any
                concurrent Pool traffic quintuples DVE op latency; a 4th
                ACT materializing z to turn the b-stt into a tt-mult saves
                ~7 us of DVE op time but costs ~11 us of extra cross-engine
                semaphore stalls — measured net loss, so 3 ACTs it is).
                The g/b stt ops fuse PAIRS of m-slots (SBUF tiles are not
                PSUM-bank-limited): the ACTs of m and m+1 write halves of
                one double-width tile, so half the stt issues / semaphore
                crossings.  Scans stay per-slot (separate recurrences).
                """
                even = (m % 2 == 0)
                if even:
                    sp = work.tile([128, 2 * tsp], BF16, tag="sp", bufs=3)
                    rp = work.tile([128, 2 * tsp], BF16, tag="rp", bufs=3)
                    a = work.tile([128, 2 * tsp], BF16, tag="a", bufs=4)
                    pair["t"] = (sp, rp, a)
                    c0 = 0
                else:
                    sp, rp, a = pair["t"]
                    c0 = tw
                nc.scalar.activation(
                    out=sp[:, c0:c0 + tw], in_=pp[:, :tw], func=AF.Sigmoid,
                    bias=bh_sb[:, m:m + 1], scale=DESCALE,
                )
                nc.scalar.activation(
                    out=rp[:, c0:c0 + tw], in_=pp[:, :tw], func=AF.Relu,
                    bias=bh_sb[:, m:m + 1], scale=DESCALE,
                )
                nc.scalar.activation(
                    out=a[:, c0:c0 + tw], in_=kp[:, :tw], func=AF.Sigmoid,
                    bias=bzn_sb[:, m:m + 1], scale=-DESCALE,
                )
                if not even:
                    w2 = 2 * tw
                    # g = min(sigmoid(p+bh), 0.5) + relu(p+bh), both halves
                    g = work.tile([128, 2 * tsp], BF16, tag="g", bufs=3)
                    nc.vector.scalar_tensor_tensor(
                        out=g[:, :w2], in0=sp[:, :w2], scalar=0.5,
                        in1=rp[:, :w2], op0=OP.min, op1=OP.add,
                    )
                    # b_neg = (a-1)*g = -z*g (z = 1-a); the scan compensates
                    # with op1=subtract: h = a*h - b_neg = a*h + z*g.
                    b = work.tile([128, 2 * tsp], BF16, tag="b", bufs=3)
                    nc.vector.scalar_tensor_tensor(
                        out=b[:, :w2], in0=a[:, :w2], scalar=1.0,
                        in1=g[:, :w2], op0=OP.subtract, op1=OP.mult,
                    )
                    pending.append((m - 1, a, b, 0, tw, ts_sl))
                    pending.append((m, a, b, tw, tw, ts_sl))

            def gate_back():
                """DVE scan + store, behind gate_front; reads its slot's
                half of the pair-fused a/b tiles."""
                m, a, b, c0, tw, ts_sl = pending.pop(0)
                # h_t = a_t * h_{t-1} + b_t along the free axis
                h = hpool.tile([128, tsp], BF16, tag=f"h{m}")
                if h_prev[m] is None:
                    init = 0.5
                else:
                    pt, pw = h_prev[m]
                    init = pt[:, pw - 1:pw]
                nc.vector.tensor_tensor_scan(
                    out=h[:, :tw], data0=a[:, c0:c0 + tw],
                    data1=b[:, c0:c0 + tw],
                    initial=init, op0=OP.mult, op1=OP.subtract,
                )
                h_prev[m] = (h, tw)
                nc.sync.dma_start(out=hT_d.ap()[m * 128:(m + 1) * 128, ts_sl],
                                  in_=h[:, :tw])

            for u, (sidx, off, tw) in enumerate(units):
                ts0 = sidx * tsp + off
                ts_sl = slice(ts0, ts0 + tw)
                if sidx == 0:
                    xs = xs0
                elif off == 0:
                    xs = xpool.tile([128, NK, tsp], F8, tag="xs")
                    nc.sync.dma_start(out=xs, in_=xp_d.ap()[:, sidx, :, :])
                # (tail sub-units reuse the strip tile loaded at off==0)
                blocks = [(off + i, min(512, tw - i)) for i in range(0, tw, 512)]
                for m in range(NM):
                    if m == 1 and not hi_loaded[0]:
                        # hi-half weights (needed from m=4, ~20 us later):
                        # triggered only now so the coalesced per-queue DMA
                        # wait in front of the first ACTs does not cover them.
                        nc.sync.dma_start(out=wh_hi, in_=whhi_d.ap())
                        nc.sync.dma_start(out=wz_hi, in_=wzhi_d.ap())
                        hi_loaded[0] = True
                    m_sl = slice((m % 4) * 128, (m % 4 + 1) * 128)
                    wz_t = wz_lo if m < 4 else wz_hi
                    wh_t = wh_lo if m < 4 else wh_hi
                    kp = psum.tile([128, tsp], F32, tag="kp")
                    pp = psum.tile([128, tsp], F32, tag="pp")
                    for wsb, out_ps in ((wh_t, pp), (wz_t, kp)):
                        for j in range(NKP):
                            ksl = slice(2 * j, 2 * j + 2)
                            for bo, bw in blocks:
                                nc.tensor.matmul(
                                    out_ps[:, bo - off:bo - off + bw],
                                    lhsT=wsb[:, ksl, m_sl],
                                    rhs=xs[:, ksl, bo:bo + bw],
                                    start=(j == 0),
                                    stop=(j == NKP - 1),
                                    perf_mode=DR,
                                )
                    if len(pending) > 0:
                        gate_back()
                    gate_front(m, kp, pp, tw, ts_sl)
            while pending:
                gate_back()

    nc.compile()
    return nc


def quantize_pack_x(x_b: np.ndarray, seq_len: int = S) -> np.ndarray:
    """x_b (seq, D) fp32 -> packed [128, nst, NK, tsp] fp8 (scaled by SX)."""
    tsp = min(1024, seq_len)
    nst = seq_len // tsp
    x8 = np.asarray(x_b * SX, dtype=NP_F8)
    return np.ascontiguousarray(
        x8.reshape(nst, tsp, NK, 128).transpose(3, 0, 2, 1)
    )


def quantize_pack_w(W: np.ndarray) -> np.ndarray:
    """W (H, D) fp32 -> packed [128, NK, H] fp8 (scaled by SW)."""
    W8 = np.asarray(W * SW, dtype=NP_F8)
    # w8[p, kt, m] = W[m, kt*128+p] * SW
    return np.ascontiguousarray(W8.T.reshape(NK, 128, H).transpose(1, 0, 2))


def make_in_maps(x, Wz, bz, Wh, bh, seq_len: int = S):
    wz8 = quantize_pack_w(np.asarray(Wz, np.float32))
    wh8 = quantize_pack_w(np.asarray(Wh, np.float32))
    bz = np.ascontiguousarray(bz, dtype=np.float32)
    bh = np.ascontiguousarray(bh, dtype=np.float32)
    halves = {
        "wz8lo": np.ascontiguousarray(wz8[:, :, :H // 2]),
        "wz8hi": np.ascontiguousarray(wz8[:, :, H // 2:]),
        "wh8lo": np.ascontiguousarray(wh8[:, :, :H // 2]),
        "wh8hi": np.ascontiguousarray(wh8[:, :, H // 2:]),
    }
    return [
        {
            "xp8": quantize_pack_x(np.asarray(x[b], np.float32), seq_len),
            **halves,
            "bzn": np.ascontiguousarray(-bz),
            "bh": bh,
        }
        for b in range(x.shape[0])
    ]


def kernel(x, Wz, bz, Wh, bh):
    x = np.ascontiguousarray(x, dtype=np.float32)
    key = "nc"
    if key not in _cache:
        _cache[key] = build_nc()
    nc = _cache[key]

    in_maps = make_in_maps(x, Wz, bz, Wh, bh)
    res = run_bass_kernel_spmd(nc, in_maps, list(range(N_CORES)))
    out = np.empty((B, S, H), np.float32)
    for b in range(N_CORES):
        out[b] = res.results[b]["hT"].astype(np.float32).T
    return out

